# revision 5
# baseline (speedup 1.0000x reference)
"""MoE post-processing MLP kernel for Trainium2 (8 NeuronCores).

Strategy: expert-parallel sharding. Each core is assigned one chunk of
samples routed to a single expert (K=8 experts ~= 8 cores for uniform
routing). The host gathers/permutes samples by expert, the device runs a
dense 3-layer MLP with the positional encoding (sin/cos) computed
on-chip, and the host scatters results back to original order.

Device kernel (per core, C=9216 samples as 9 iterations of 1024):
  - samples pair-packed: tile [78, 512] = two 39-row blocks (feat32 +
    pos3 + view3 + ones) for 2x512 samples
  - u = Rdd^T @ fpv  (scaled angles / 2pi, phase folded via ones row)
  - r = round(u) via magic-constant add/sub on VectorE
  - u -= r via accumulating -I matmul (range reduction mod 1)
  - s36 = sin(2*pi*u) on ScalarE (LUT valid within [-pi, pi])
  - h0 = relu(W0a^T@fpv + W0s^T@s36 + b0); h1 = relu(W1^T@h0 + b1)
  - y = W2^T@h1 + b2  (block-diagonal weights process both halves)
All matmuls run in float32r (full-rate fp32 mode on the PE).
"""

import numpy as np

K = 8
WID = 64
D = 32
NT = 512            # matmul moving free dim (one PSUM bank of fp32)
NITER = 9           # iterations per invocation
C = NITER * 2 * NT  # 9216 samples per core-chunk
CMAGIC = 12582912.0  # 1.5 * 2**23, round-to-nearest magic constant

# W0 row indices (DIN=74 layout: feat 0:32, posenc(pos,2) 32:47,
# posenc(view,4) 47:74) for the identity part and the sin part.
_W0A_ROWS = list(range(32)) + [32, 33, 34] + [47, 48, 49]
_W0S_ROWS = (list(range(35, 41)) + list(range(50, 62))
             + list(range(41, 47)) + list(range(62, 74)))

_PREP = None  # (nc, input_names) built once per process
_LAST_IN_MAPS = None  # stashed for external profiling harnesses


def _build_R():
    """R' [7, 36]: u = (scale*x + phase)/(2pi); rows = [p0..2, v0..2, one]."""
    Rp = np.zeros((7, 36), np.float32)
    col = 0
    for phase in range(2):
        for base, scales in ((0, [1.0, 2.0]), (3, [1.0, 2.0, 4.0, 8.0])):
            for m in scales:
                for c in range(3):
                    Rp[base + c, col] = m / (2 * np.pi)
                    Rp[6, col] = 0.25 * phase
                    col += 1
    return Rp


def _build_program():
    import concourse.bacc as bacc
    import concourse.mybir as mybir
    from concourse.tile import TileContext

    F32, F32R = mybir.dt.float32, mybir.dt.float32r
    AF = mybir.ActivationFunctionType
    ALU = mybir.AluOpType

    nc = bacc.Bacc("TRN2", target_bir_lowering=False, debug=False,
                   num_devices=8)

    def din(name, shape, dt=F32R):
        return nc.dram_tensor(name, list(shape), dt, kind="ExternalInput").ap()

    fpv_d = din("fpv", [78, C // 2])
    R_d = din("Rdd", [78, 72])
    I_d = din("I72n", [72, 72])
    W0a_d = din("W0add", [78, 128])
    W0s_d = din("W0sdd", [72, 128])
    W1_d = din("W1dd", [128, 128])
    W2_d = din("W2dd", [128, 64])
    b0_d = din("b0", [128, 1], F32)
    b1_d = din("b1", [128, 1], F32)
    b2_d = din("b2", [64, 1], F32)
    y_d = nc.dram_tensor("y", [64, C // 2], F32, kind="ExternalOutput").ap()

    with TileContext(nc) as tc:
        with (tc.tile_pool(name="w", bufs=1) as wp,
              tc.tile_pool(name="io", bufs=3) as io,
              tc.tile_pool(name="ps", bufs=2, space="PSUM") as ps):
            Rt = wp.tile([78, 72], F32R)
            nc.sync.dma_start(out=Rt[:], in_=R_d[:])
            It = wp.tile([72, 72], F32R)
            nc.sync.dma_start(out=It[:], in_=I_d[:])
            W0at = wp.tile([78, 128], F32R)
            nc.sync.dma_start(out=W0at[:], in_=W0a_d[:])
            W0st = wp.tile([72, 128], F32R)
            nc.sync.dma_start(out=W0st[:], in_=W0s_d[:])
            W1t = wp.tile([128, 128], F32R)
            nc.sync.dma_start(out=W1t[:], in_=W1_d[:])
            W2t = wp.tile([128, 64], F32R)
            nc.sync.dma_start(out=W2t[:], in_=W2_d[:])
            b0t = wp.tile([128, 1], F32)
            nc.sync.dma_start(out=b0t[:], in_=b0_d[:])
            b1t = wp.tile([128, 1], F32)
            nc.sync.dma_start(out=b1t[:], in_=b1_d[:])
            b2t = wp.tile([64, 1], F32)
            nc.sync.dma_start(out=b2t[:], in_=b2_d[:])

            for i in range(NITER):
                cs = slice(i * NT, (i + 1) * NT)
                fpvt = io.tile([78, NT], F32R)
                nc.sync.dma_start(out=fpvt[:], in_=fpv_d[:, cs])

                up = ps.tile([72, NT], F32)
                nc.tensor.matmul(out=up[:], lhsT=Rt[:], rhs=fpvt[:],
                                 start=True, stop=False)
                rt = io.tile([72, NT], F32R)
                nc.vector.tensor_scalar(out=rt[:], in0=up[:], scalar1=CMAGIC,
                                        scalar2=CMAGIC, op0=ALU.add,
                                        op1=ALU.subtract)
                nc.tensor.matmul(out=up[:], lhsT=It[:], rhs=rt[:],
                                 start=False, stop=True)
                s36t = io.tile([72, NT], F32R)
                nc.scalar.activation(s36t[:], up[:], AF.Sin, bias=0.0,
                                     scale=float(2 * np.pi))

                h0p = ps.tile([128, NT], F32)
                nc.tensor.matmul(out=h0p[:], lhsT=W0at[:], rhs=fpvt[:],
                                 start=True, stop=False)
                nc.tensor.matmul(out=h0p[:], lhsT=W0st[:], rhs=s36t[:],
                                 start=False, stop=True)
                h0t = io.tile([128, NT], F32R)
                nc.scalar.activation(h0t[:], h0p[:], AF.Relu, bias=b0t[:],
                                     scale=1.0)

                h1p = ps.tile([128, NT], F32)
                nc.tensor.matmul(out=h1p[:], lhsT=W1t[:], rhs=h0t[:],
                                 start=True, stop=True)
                h1t = io.tile([128, NT], F32R)
                nc.scalar.activation(h1t[:], h1p[:], AF.Relu, bias=b1t[:],
                                     scale=1.0)

                yp = ps.tile([64, NT], F32)
                nc.tensor.matmul(out=yp[:], lhsT=W2t[:], rhs=h1t[:],
                                 start=True, stop=True)
                yt = io.tile([64, NT], F32)
                nc.vector.tensor_scalar(out=yt[:], in0=yp[:], scalar1=b2t[:],
                                        scalar2=None, op0=ALU.add)
                nc.sync.dma_start(out=y_d[:, cs], in_=yt[:])

    nc.compile()
    return nc


def _get_program():
    global _PREP
    if _PREP is None:
        _PREP = _build_program()
    return _PREP


def _pack_weights(W0, b0, W1, b1, W2, b2):
    """Per-expert block-diagonal device weight arrays."""
    W0a = np.zeros((39, 64), np.float32)
    W0a[0:38] = W0[_W0A_ROWS]
    W0s = W0[_W0S_ROWS].astype(np.float32)

    Rp = _build_R()
    Rdd = np.zeros((78, 72), np.float32)
    Rdd[32:39, 0:36] = Rp
    Rdd[71:78, 36:72] = Rp
    W0add = np.zeros((78, 128), np.float32)
    W0add[0:39, 0:64] = W0a
    W0add[39:78, 64:128] = W0a
    W0sdd = np.zeros((72, 128), np.float32)
    W0sdd[0:36, 0:64] = W0s
    W0sdd[36:72, 64:128] = W0s
    W1dd = np.zeros((128, 128), np.float32)
    W1dd[0:64, 0:64] = W1
    W1dd[64:128, 64:128] = W1
    W2dd = np.zeros((128, 64), np.float32)
    W2dd[0:64, 0:32] = W2
    W2dd[64:128, 32:64] = W2
    return {
        "Rdd": Rdd, "I72n": -np.eye(72, dtype=np.float32),
        "W0add": W0add, "W0sdd": W0sdd, "W1dd": W1dd, "W2dd": W2dd,
        "b0": np.concatenate([b0, b0]).reshape(128, 1).astype(np.float32),
        "b1": np.concatenate([b1, b1]).reshape(128, 1).astype(np.float32),
        "b2": np.concatenate([b2, b2]).reshape(64, 1).astype(np.float32),
    }


def kernel(idxs, positions, viewdirs, features, W0, b0, W1, b1, W2, b2):
    from concourse.bass_utils import run_bass_kernel_spmd

    N = idxs.shape[0]
    idx = idxs.reshape(-1).astype(np.int64)
    out = np.zeros((N, D), np.float32)

    # Route: list of (expert, sample-index-array) chunks of <= C samples.
    chunks = []
    for k in range(K):
        sel = np.nonzero(idx == k)[0]
        for s in range(0, max(len(sel), 1)):
            lo = s * C
            if lo >= len(sel) and not (lo == 0 and len(sel) == 0):
                break
            if len(sel) == 0:
                break
            chunks.append((k, sel[lo:lo + C]))
            if lo + C >= len(sel):
                break

    wpacks = [_pack_weights(W0[k], b0[k], W1[k], b1[k], W2[k], b2[k])
              for k in range(K)]

    nc = _get_program()
    zero_in = None
    for inv in range(0, len(chunks), 8):
        batch = chunks[inv:inv + 8]
        in_maps = []
        for ci in range(8):
            if ci < len(batch):
                k, sel = batch[ci]
                n = len(sel)
                fpv39 = np.zeros((39, C), np.float32)
                fpv39[0:32, :n] = features[sel].T
                fpv39[32:35, :n] = positions[sel].T
                fpv39[35:38, :n] = viewdirs[sel].T
                fpv39[38, :] = 1.0
                fpv78 = np.concatenate(
                    [fpv39.reshape(39, NITER, 2, NT)[:, :, 0],
                     fpv39.reshape(39, NITER, 2, NT)[:, :, 1]],
                    axis=0).reshape(78, C // 2)
                m = dict(wpacks[k])
                m["fpv"] = np.ascontiguousarray(fpv78)
                in_maps.append(m)
            else:
                if zero_in is None:
                    zero_in = dict(wpacks[0])
                    zero_in["fpv"] = np.zeros((78, C // 2), np.float32)
                in_maps.append(zero_in)
        global _LAST_IN_MAPS
        _LAST_IN_MAPS = in_maps
        res = None
        for attempt in range(3):
            try:
                res = run_bass_kernel_spmd(nc, in_maps,
                                           core_ids=list(range(8)))
                break
            except Exception:
                if attempt == 2:
                    raise
        assert res is not None
        for ci, (k, sel) in enumerate(batch):
            y64 = res.results[ci]["y"]                   # [64, C//2]
            y32 = np.stack([y64[0:32].reshape(D, NITER, NT),
                            y64[32:64].reshape(D, NITER, NT)],
                           axis=2).reshape(D, C)
            out[sel] = y32[:, :len(sel)].T
    return out
